# revision 10
# baseline (speedup 1.0000x reference)
"""DistMult edge scoring on 8 Trainium2 NeuronCores.

score[e] = sigmoid(sum_d h[u[e],d] * rel_weight[etype[e],d] * h[v[e],d])

Strategy (v4: per-core gather tables, non-transpose gathers, DVE tree reduce)
-----------------------------------------------------------------------------
Edges are sharded across the 8 cores by u-range (core c takes u in
[12500c, 12500(c+1))) and each core's slot count is equalized (max over
cores, rounded to 128) so one SPMD program serves all cores.

Both gather sides read from small per-core tables built on the host:
  TU[j] = h[u_j] * rel_weight[k_j]   for the core's distinct (etype, u)
                                     pairs (the relation weight is folded
                                     into the u side),
  TV[j] = h[v_j]                     for the core's distinct v values.
Both tables have < 32768 rows, so slot indices are plain int16 table
ordinals -- no v-windowing, no per-etype runs, and only ~50 MB of DRAM
per core.  Slots are sorted by (v, u), making the v-side gather read
ascending (often repeated) rows -- HBM-friendly -- while the u side hits
a small (~25 MB) table.

Per 4096-slot chunk the kernel issues two *non-transposed* `dma_gather`s
(single_packet=False: coalescing ~260 descriptors into one packet
exceeds the <=64-descriptor packet ceiling and wedges the DMA).  With no
xbar in the non-transpose path, gathers rotate across all 4 SWDGE queues
so descriptor generation runs on up to 4 Q7 core pairs in parallel (it
was the serialized ~570us bottleneck of the transposed v2 design).

Gathered rows land edge-major [128, cols, 384].  One DVE fp16 multiply
(2x mode) forms prod = huR * hv; a binary-tree reduce over d via three
fp16 tensor_tensor adds (2x: 384->192->96->48) plus one 48->1
TensorReduce into fp32 (TensorReduce supports no 2x mode, so the tree
does most of the work at double rate) yields per-slot scores.  ACT
applies the sigmoid into a persistent [128, tot/128] tile DMA'd out
once at the end.
"""

import numpy as np

import concourse.bacc as bacc
import concourse.mybir as mybir
import concourse.tile as tile
from concourse.bass_utils import run_bass_kernel_spmd

N_NODES = 100000
D = 384
N_ETYPES = 8
N_CORES = 8
USHARD = N_NODES // N_CORES   # 12500 u-rows per core
SLOTPAD = 128              # slot-count granularity (layout unit)
TCAP = 32768               # gather-table rows (>= max distinct ids per core)
CH = 2048                  # slots per gather chunk
NQ = 4                     # SWDGE queues (desc-gen core pairs)

_cache = {}


def _build(tot):
    f16 = mybir.dt.float16
    f32 = mybir.dt.float32
    assert tot % SLOTPAD == 0
    totc = tot // 128

    nc = bacc.Bacc(
        "TRN2",
        target_bir_lowering=False,
        debug=False,
        enable_asserts=False,
        num_devices=N_CORES,
        num_swdge_queues=NQ,
    )
    tu_ap = nc.dram_tensor("tu", [TCAP, D], f16, kind="ExternalInput").ap()
    tv_ap = nc.dram_tensor("tv", [TCAP, D], f16, kind="ExternalInput").ap()
    uidx = nc.dram_tensor("uidx", [128, tot // 16], mybir.dt.int16, kind="ExternalInput").ap()
    vidx = nc.dram_tensor("vidx", [128, tot // 16], mybir.dt.int16, kind="ExternalInput").ap()
    out = nc.dram_tensor("out", [128, totc], f32, kind="ExternalOutput").ap()

    chunks = []
    pos = 0
    while pos < tot:
        n = min(CH, tot - pos)
        chunks.append((pos, n))
        pos += n

    q = 0
    with tile.TileContext(nc) as tc:
        with (
            tc.tile_pool(name="const", bufs=1) as cpool,
            tc.tile_pool(name="gath", bufs=4) as gpool,
            tc.tile_pool(name="prod", bufs=3) as wpool,
            tc.tile_pool(name="red", bufs=3) as rpool,
        ):
            u_sb = cpool.tile([128, tot // 16], mybir.dt.int16)
            v_sb = cpool.tile([128, tot // 16], mybir.dt.int16)
            scores = cpool.tile([128, totc], f32)

            for (off, n) in chunks:
                cols = n // 128
                c0, c1 = off // 16, (off + n) // 16
                nc.sync.dma_start(out=u_sb[:, c0:c1], in_=uidx[:, c0:c1])
                nc.sync.dma_start(out=v_sb[:, c0:c1], in_=vidx[:, c0:c1])

                hu = gpool.tile([128, cols * D], f16, tag="hu")
                nc.gpsimd.dma_gather(
                    hu[:].rearrange("p (s d) -> p s d", d=D),
                    tu_ap[:],
                    u_sb[:, off // 16 : (off + n) // 16],
                    n, n, D,
                    transpose=False,
                    single_packet=False,
                    queue_num=q % NQ,
                )
                q += 1
                hv = gpool.tile([128, cols * D], f16, tag="hv")
                nc.gpsimd.dma_gather(
                    hv[:].rearrange("p (s d) -> p s d", d=D),
                    tv_ap[:],
                    v_sb[:, off // 16 : (off + n) // 16],
                    n, n, D,
                    transpose=False,
                    single_packet=False,
                    queue_num=q % NQ,
                )
                q += 1

                pr = wpool.tile([128, cols * D], f16, tag="pr")
                nc.vector.tensor_mul(out=pr[:], in0=hu[:], in1=hv[:])
                # binary-tree reduce over d at 2x: 384 -> 192 -> 96 -> 48 (fp16)
                with nc.allow_low_precision("fp16 tree partial sums (3 rounding levels)"):
                    h1 = rpool.tile([128, cols * (D // 2)], f16, tag="h1")
                    p3 = pr[:].rearrange("p (s d) -> p s d", d=D)
                    nc.vector.tensor_add(
                        out=h1[:].rearrange("p (s d) -> p s d", d=D // 2),
                        in0=p3[:, :, 0 : D // 2],
                        in1=p3[:, :, D // 2 : D],
                    )
                    h2 = rpool.tile([128, cols * (D // 4)], f16, tag="h2")
                    h1r = h1[:].rearrange("p (s d) -> p s d", d=D // 2)
                    nc.vector.tensor_add(
                        out=h2[:].rearrange("p (s d) -> p s d", d=D // 4),
                        in0=h1r[:, :, 0 : D // 4],
                        in1=h1r[:, :, D // 4 : D // 2],
                    )
                    h3 = rpool.tile([128, cols * (D // 8)], f16, tag="h3")
                    h2r = h2[:].rearrange("p (s d) -> p s d", d=D // 4)
                    nc.vector.tensor_add(
                        out=h3[:].rearrange("p (s d) -> p s d", d=D // 8),
                        in0=h2r[:, :, 0 : D // 8],
                        in1=h2r[:, :, D // 8 : D // 4],
                    )
                sc = rpool.tile([128, cols], f32, tag="sc")
                nc.vector.tensor_reduce(
                    out=sc[:],
                    in_=h3[:].rearrange("p (s d) -> p s d", d=D // 8),
                    axis=mybir.AxisListType.X,
                    op=mybir.AluOpType.add,
                )
                nc.scalar.activation(
                    out=scores[:, off // 128 : off // 128 + cols],
                    in_=sc[:],
                    func=mybir.ActivationFunctionType.Sigmoid,
                )

            nc.sync.dma_start(out=out[:], in_=scores[:])

    nc.compile()
    return nc


def _get_nc(tot):
    if tot not in _cache:
        _cache[tot] = _build(tot)
    return _cache[tot]


def _wrap16(a):
    """[n] int16 -> [128, n/16] wrapped over 16 partitions, replicated 8x."""
    n = a.shape[0]
    return np.tile(a.reshape(n // 16, 16).T, (8, 1))


def _shard(u32, v32, et):
    """Shard edges by u-range; per core sort by (v, u) and build int16
    table-ordinal slot indices for the (etype,u)-pair and distinct-v
    gather tables."""
    core = u32 // USHARD
    per_core_ids = []
    max_edges = 0
    for c in range(N_CORES):
        ids = np.nonzero(core == c)[0]
        order = np.lexsort((u32[ids], v32[ids]))
        ids = ids[order]
        max_edges = max(max_edges, ids.shape[0])
        per_core_ids.append(ids)
    tot = (max_edges + SLOTPAD - 1) // SLOTPAD * SLOTPAD

    per_core = []
    for c in range(N_CORES):
        ids = per_core_ids[c]
        ne = ids.shape[0]
        # u side: distinct (etype, u) pairs
        pkey = et[ids] * N_NODES + u32[ids]
        pairs, pinv = np.unique(pkey, return_inverse=True)
        assert pairs.shape[0] <= TCAP
        pu = (pairs % N_NODES).astype(np.int64)
        pk = (pairs // N_NODES).astype(np.int64)
        # v side: distinct v
        vvals, vinv = np.unique(v32[ids], return_inverse=True)
        assert vvals.shape[0] <= TCAP

        u_slots = np.zeros(tot, np.int16)
        v_slots = np.zeros(tot, np.int16)
        eid = np.full(tot, -1, np.int64)
        u_slots[:ne] = pinv.astype(np.int16)
        v_slots[:ne] = vinv.astype(np.int16)
        eid[:ne] = ids
        per_core.append((u_slots, v_slots, eid, pu, pk, vvals.astype(np.int64)))
    return tot, per_core


def _make_in_maps(h, rel_weight, per_core):
    h32 = np.asarray(h, np.float32)
    rel32 = np.asarray(rel_weight, np.float32)
    in_maps = []
    for c in range(N_CORES):
        u_slots, v_slots, _eid, pu, pk, vvals = per_core[c]
        tu = np.zeros((TCAP, D), np.float16)
        tu[: pu.shape[0]] = (h32[pu] * rel32[pk]).astype(np.float16)
        tv = np.zeros((TCAP, D), np.float16)
        tv[: vvals.shape[0]] = h32[vvals].astype(np.float16)
        in_maps.append(
            {
                "tu": tu,
                "tv": tv,
                "uidx": np.ascontiguousarray(_wrap16(u_slots)),
                "vidx": np.ascontiguousarray(_wrap16(v_slots)),
            }
        )
    return in_maps


def run_spmd(h, u, v, etype, rel_weight, trace=False, trace_cores=None):
    """Run the SPMD kernel; returns (full_output, BassKernelResults)."""
    u32 = np.asarray(u, np.int64).astype(np.int32)
    v32 = np.asarray(v, np.int64).astype(np.int32)
    et = np.asarray(etype, np.int64)
    n_edges = u32.shape[0]

    tot, per_core = _shard(u32, v32, et)
    nc = _get_nc(tot)
    in_maps = _make_in_maps(h, rel_weight, per_core)
    res = run_bass_kernel_spmd(
        nc,
        in_maps,
        core_ids=list(range(N_CORES)),
        trace=trace,
        trace_cores=trace_cores,
    )
    result = np.zeros(n_edges, np.float32)
    for c in range(N_CORES):
        o = np.asarray(res.results[c]["out"])  # [128, tot/128] f32
        vals = o.T.reshape(-1)                 # vals[s] = o[s % 128, s // 128]
        eid = per_core[c][2]
        m = eid >= 0
        result[eid[m]] = vals[m]
    return result, res


def kernel(h, u, v, etype, rel_weight):
    out, _ = run_spmd(h, u, v, etype, rel_weight)
    return out


# revision 11
# speedup vs baseline: 1.0467x; 1.0467x over previous
"""DistMult edge scoring on 8 Trainium2 NeuronCores.

score[e] = sigmoid(sum_d h[u[e],d] * rel_weight[etype[e],d] * h[v[e],d])

Strategy (v4: per-core gather tables, non-transpose gathers, DVE tree reduce)
-----------------------------------------------------------------------------
Edges are sharded across the 8 cores by u-range (core c takes u in
[12500c, 12500(c+1))) and each core's slot count is equalized (max over
cores, rounded to 128) so one SPMD program serves all cores.

Both gather sides read from small per-core tables built on the host:
  TU[j] = h[u_j] * rel_weight[k_j]   for the core's distinct (etype, u)
                                     pairs (the relation weight is folded
                                     into the u side),
  TV[j] = h[v_j]                     for the core's distinct v values.
Both tables have < 32768 rows, so slot indices are plain int16 table
ordinals -- no v-windowing, no per-etype runs, and only ~50 MB of DRAM
per core.  Slots are sorted by (v, u), making the v-side gather read
ascending (often repeated) rows -- HBM-friendly -- while the u side hits
a small (~25 MB) table.

Per 4096-slot chunk the kernel issues two *non-transposed* `dma_gather`s
(single_packet=False: coalescing ~260 descriptors into one packet
exceeds the <=64-descriptor packet ceiling and wedges the DMA).  With no
xbar in the non-transpose path, gathers rotate across all 4 SWDGE queues
so descriptor generation runs on up to 4 Q7 core pairs in parallel (it
was the serialized ~570us bottleneck of the transposed v2 design).

Gathered rows land edge-major [128, cols, 384].  One DVE fp16 multiply
(2x mode) forms prod = huR * hv; a binary-tree reduce over d via three
fp16 tensor_tensor adds (2x: 384->192->96->48) plus one 48->1
TensorReduce into fp32 (TensorReduce supports no 2x mode, so the tree
does most of the work at double rate) yields per-slot scores.  ACT
applies the sigmoid into a persistent [128, tot/128] tile DMA'd out
once at the end.
"""

import numpy as np

import concourse.bacc as bacc
import concourse.mybir as mybir
import concourse.tile as tile
from concourse.bass_utils import run_bass_kernel_spmd

N_NODES = 100000
D = 384
N_ETYPES = 8
N_CORES = 8
USHARD = N_NODES // N_CORES   # 12500 u-rows per core
SLOTPAD = 128              # slot-count granularity (layout unit)
TCAP = 32768               # gather-table rows (>= max distinct ids per core)
CH = 2048                  # slots per gather chunk
NQ = 4                     # SWDGE queues (desc-gen core pairs)

_cache = {}


def _build(tot):
    f16 = mybir.dt.float16
    f32 = mybir.dt.float32
    assert tot % SLOTPAD == 0
    totc = tot // 128

    nc = bacc.Bacc(
        "TRN2",
        target_bir_lowering=False,
        debug=False,
        enable_asserts=False,
        num_devices=N_CORES,
        num_swdge_queues=NQ,
    )
    tu_ap = nc.dram_tensor("tu", [TCAP, D], f16, kind="ExternalInput").ap()
    tv_ap = nc.dram_tensor("tv", [TCAP, D], f16, kind="ExternalInput").ap()
    uidx = nc.dram_tensor("uidx", [128, tot // 16], mybir.dt.int16, kind="ExternalInput").ap()
    vidx = nc.dram_tensor("vidx", [128, tot // 16], mybir.dt.int16, kind="ExternalInput").ap()
    out = nc.dram_tensor("out", [128, totc], f32, kind="ExternalOutput").ap()

    chunks = []
    pos = 0
    while pos < tot:
        n = min(CH, tot - pos)
        chunks.append((pos, n))
        pos += n

    q = 0
    with tile.TileContext(nc) as tc:
        with (
            tc.tile_pool(name="const", bufs=1) as cpool,
            tc.tile_pool(name="gath", bufs=4) as gpool,
            tc.tile_pool(name="prod", bufs=3) as wpool,
            tc.tile_pool(name="red", bufs=3) as rpool,
        ):
            u_sb = cpool.tile([128, tot // 16], mybir.dt.int16)
            nc.sync.dma_start(out=u_sb[:], in_=uidx[:])
            v_sb = cpool.tile([128, tot // 16], mybir.dt.int16)
            nc.sync.dma_start(out=v_sb[:], in_=vidx[:])
            scores = cpool.tile([128, totc], f32)

            for (off, n) in chunks:
                cols = n // 128

                hu = gpool.tile([128, cols * D], f16, tag="hu")
                nc.gpsimd.dma_gather(
                    hu[:].rearrange("p (s d) -> p s d", d=D),
                    tu_ap[:],
                    u_sb[:, off // 16 : (off + n) // 16],
                    n, n, D,
                    transpose=False,
                    single_packet=False,
                    queue_num=q % NQ,
                )
                q += 1
                hv = gpool.tile([128, cols * D], f16, tag="hv")
                nc.gpsimd.dma_gather(
                    hv[:].rearrange("p (s d) -> p s d", d=D),
                    tv_ap[:],
                    v_sb[:, off // 16 : (off + n) // 16],
                    n, n, D,
                    transpose=False,
                    single_packet=False,
                    queue_num=q % NQ,
                )
                q += 1

                pr = wpool.tile([128, cols * D], f16, tag="pr")
                nc.vector.tensor_mul(out=pr[:], in0=hu[:], in1=hv[:])
                # binary-tree reduce over d at 2x: 384 -> 192 -> 96 -> 48 (fp16)
                with nc.allow_low_precision("fp16 tree partial sums (3 rounding levels)"):
                    h1 = rpool.tile([128, cols * (D // 2)], f16, tag="h1")
                    p3 = pr[:].rearrange("p (s d) -> p s d", d=D)
                    nc.vector.tensor_add(
                        out=h1[:].rearrange("p (s d) -> p s d", d=D // 2),
                        in0=p3[:, :, 0 : D // 2],
                        in1=p3[:, :, D // 2 : D],
                    )
                    h2 = rpool.tile([128, cols * (D // 4)], f16, tag="h2")
                    h1r = h1[:].rearrange("p (s d) -> p s d", d=D // 2)
                    nc.vector.tensor_add(
                        out=h2[:].rearrange("p (s d) -> p s d", d=D // 4),
                        in0=h1r[:, :, 0 : D // 4],
                        in1=h1r[:, :, D // 4 : D // 2],
                    )
                    h3 = rpool.tile([128, cols * (D // 8)], f16, tag="h3")
                    h2r = h2[:].rearrange("p (s d) -> p s d", d=D // 4)
                    nc.vector.tensor_add(
                        out=h3[:].rearrange("p (s d) -> p s d", d=D // 8),
                        in0=h2r[:, :, 0 : D // 8],
                        in1=h2r[:, :, D // 8 : D // 4],
                    )
                sc = rpool.tile([128, cols], f32, tag="sc")
                nc.vector.tensor_reduce(
                    out=sc[:],
                    in_=h3[:].rearrange("p (s d) -> p s d", d=D // 8),
                    axis=mybir.AxisListType.X,
                    op=mybir.AluOpType.add,
                )
                nc.scalar.activation(
                    out=scores[:, off // 128 : off // 128 + cols],
                    in_=sc[:],
                    func=mybir.ActivationFunctionType.Sigmoid,
                )

            nc.sync.dma_start(out=out[:], in_=scores[:])

    nc.compile()
    return nc


def _get_nc(tot):
    if tot not in _cache:
        _cache[tot] = _build(tot)
    return _cache[tot]


def _wrap16(a):
    """[n] int16 -> [128, n/16] wrapped over 16 partitions, replicated 8x."""
    n = a.shape[0]
    return np.tile(a.reshape(n // 16, 16).T, (8, 1))


def _shard(u32, v32, et):
    """Shard edges by u-range; per core sort by (v, u) and build int16
    table-ordinal slot indices for the (etype,u)-pair and distinct-v
    gather tables."""
    core = u32 // USHARD
    per_core_ids = []
    max_edges = 0
    for c in range(N_CORES):
        ids = np.nonzero(core == c)[0]
        order = np.lexsort((u32[ids], v32[ids]))
        ids = ids[order]
        max_edges = max(max_edges, ids.shape[0])
        per_core_ids.append(ids)
    tot = (max_edges + SLOTPAD - 1) // SLOTPAD * SLOTPAD

    per_core = []
    for c in range(N_CORES):
        ids = per_core_ids[c]
        ne = ids.shape[0]
        # u side: distinct (etype, u) pairs
        pkey = et[ids] * N_NODES + u32[ids]
        pairs, pinv = np.unique(pkey, return_inverse=True)
        assert pairs.shape[0] <= TCAP
        # reorder table rows by first use in slot order so uidx is nearly
        # ascending -> the u-side gather reads the table near-sequentially
        npair = pairs.shape[0]
        first_pos = np.full(npair, np.iinfo(np.int64).max, np.int64)
        np.minimum.at(first_pos, pinv, np.arange(ne, dtype=np.int64))
        order = np.argsort(first_pos, kind="stable")
        rank = np.empty(npair, np.int64)
        rank[order] = np.arange(npair)
        pinv = rank[pinv]
        pairs = pairs[order]
        pu = (pairs % N_NODES).astype(np.int64)
        pk = (pairs // N_NODES).astype(np.int64)
        # v side: distinct v
        vvals, vinv = np.unique(v32[ids], return_inverse=True)
        assert vvals.shape[0] <= TCAP

        u_slots = np.zeros(tot, np.int16)
        v_slots = np.zeros(tot, np.int16)
        eid = np.full(tot, -1, np.int64)
        u_slots[:ne] = pinv.astype(np.int16)
        v_slots[:ne] = vinv.astype(np.int16)
        eid[:ne] = ids
        per_core.append((u_slots, v_slots, eid, pu, pk, vvals.astype(np.int64)))
    return tot, per_core


def _make_in_maps(h, rel_weight, per_core):
    h32 = np.asarray(h, np.float32)
    rel32 = np.asarray(rel_weight, np.float32)
    in_maps = []
    for c in range(N_CORES):
        u_slots, v_slots, _eid, pu, pk, vvals = per_core[c]
        tu = np.zeros((TCAP, D), np.float16)
        tu[: pu.shape[0]] = (h32[pu] * rel32[pk]).astype(np.float16)
        tv = np.zeros((TCAP, D), np.float16)
        tv[: vvals.shape[0]] = h32[vvals].astype(np.float16)
        in_maps.append(
            {
                "tu": tu,
                "tv": tv,
                "uidx": np.ascontiguousarray(_wrap16(u_slots)),
                "vidx": np.ascontiguousarray(_wrap16(v_slots)),
            }
        )
    return in_maps


def run_spmd(h, u, v, etype, rel_weight, trace=False, trace_cores=None):
    """Run the SPMD kernel; returns (full_output, BassKernelResults)."""
    u32 = np.asarray(u, np.int64).astype(np.int32)
    v32 = np.asarray(v, np.int64).astype(np.int32)
    et = np.asarray(etype, np.int64)
    n_edges = u32.shape[0]

    tot, per_core = _shard(u32, v32, et)
    nc = _get_nc(tot)
    in_maps = _make_in_maps(h, rel_weight, per_core)
    res = run_bass_kernel_spmd(
        nc,
        in_maps,
        core_ids=list(range(N_CORES)),
        trace=trace,
        trace_cores=trace_cores,
    )
    result = np.zeros(n_edges, np.float32)
    for c in range(N_CORES):
        o = np.asarray(res.results[c]["out"])  # [128, tot/128] f32
        vals = o.T.reshape(-1)                 # vals[s] = o[s % 128, s // 128]
        eid = per_core[c][2]
        m = eid >= 0
        result[eid[m]] = vals[m]
    return result, res


def kernel(h, u, v, etype, rel_weight):
    out, _ = run_spmd(h, u, v, etype, rel_weight)
    return out


# revision 12
# speedup vs baseline: 1.0683x; 1.0206x over previous
"""DistMult edge scoring on 8 Trainium2 NeuronCores.

score[e] = sigmoid(sum_d h[u[e],d] * rel_weight[etype[e],d] * h[v[e],d])

Strategy (v4: per-core gather tables, non-transpose gathers, DVE tree reduce)
-----------------------------------------------------------------------------
Edges are sharded across the 8 cores by u-range (core c takes u in
[12500c, 12500(c+1))) and each core's slot count is equalized (max over
cores, rounded to 128) so one SPMD program serves all cores.

Both gather sides read from small per-core tables built on the host:
  TU[j] = h[u_j] * rel_weight[k_j]   for the core's distinct (etype, u)
                                     pairs (the relation weight is folded
                                     into the u side),
  TV[j] = h[v_j]                     for the core's distinct v values.
Both tables have < 32768 rows, so slot indices are plain int16 table
ordinals -- no v-windowing, no per-etype runs, and only ~50 MB of DRAM
per core.  Slots are sorted by (v, u), making the v-side gather read
ascending (often repeated) rows -- HBM-friendly -- while the u side hits
a small (~25 MB) table.

Per 4096-slot chunk the kernel issues two *non-transposed* `dma_gather`s
(single_packet=False: coalescing ~260 descriptors into one packet
exceeds the <=64-descriptor packet ceiling and wedges the DMA).  With no
xbar in the non-transpose path, gathers rotate across all 4 SWDGE queues
so descriptor generation runs on up to 4 Q7 core pairs in parallel (it
was the serialized ~570us bottleneck of the transposed v2 design).

Gathered rows land edge-major [128, cols, 384].  One DVE fp16 multiply
(2x mode) forms prod = huR * hv; a binary-tree reduce over d via three
fp16 tensor_tensor adds (2x: 384->192->96->48) plus one 48->1
TensorReduce into fp32 (TensorReduce supports no 2x mode, so the tree
does most of the work at double rate) yields per-slot scores.  ACT
applies the sigmoid into a persistent [128, tot/128] tile DMA'd out
once at the end.
"""

import numpy as np

import concourse.bacc as bacc
import concourse.mybir as mybir
import concourse.tile as tile
from concourse.bass_utils import run_bass_kernel_spmd

N_NODES = 100000
D = 384
N_ETYPES = 8
N_CORES = 8
USHARD = N_NODES // N_CORES   # 12500 u-rows per core
SLOTPAD = 128              # slot-count granularity (layout unit)
TCAP = 32768               # gather-table rows (>= max distinct ids per core)
CH = 2048                  # slots per gather chunk
NQ = 4                     # SWDGE queues (desc-gen core pairs)

_cache = {}


def _build(tot):
    f16 = mybir.dt.float16
    f32 = mybir.dt.float32
    assert tot % SLOTPAD == 0
    totc = tot // 128

    nc = bacc.Bacc(
        "TRN2",
        target_bir_lowering=False,
        debug=False,
        enable_asserts=False,
        num_devices=N_CORES,
        num_swdge_queues=NQ,
    )
    tu_ap = nc.dram_tensor("tu", [TCAP, D], f16, kind="ExternalInput").ap()
    tv_ap = nc.dram_tensor("tv", [TCAP, D], f16, kind="ExternalInput").ap()
    uidx = nc.dram_tensor("uidx", [128, tot // 16], mybir.dt.int16, kind="ExternalInput").ap()
    vidx = nc.dram_tensor("vidx", [128, tot // 16], mybir.dt.int16, kind="ExternalInput").ap()
    out = nc.dram_tensor("out", [128, totc], f32, kind="ExternalOutput").ap()

    chunks = []
    pos = 0
    while pos < tot:
        n = min(CH, tot - pos)
        chunks.append((pos, n))
        pos += n

    q = 0
    with tile.TileContext(nc) as tc:
        with (
            tc.tile_pool(name="const", bufs=1) as cpool,
            tc.tile_pool(name="gath", bufs=4) as gpool,
            tc.tile_pool(name="prod", bufs=3) as wpool,
            tc.tile_pool(name="red", bufs=3) as rpool,
        ):
            # idx preloads split into a small head (first 2 chunks) + the
            # remainder so the first gathers' desc-gen starts immediately
            hc = min(2 * CH, tot) // 16
            u_sb = cpool.tile([128, tot // 16], mybir.dt.int16)
            nc.sync.dma_start(out=u_sb[:, 0:hc], in_=uidx[:, 0:hc])
            v_sb = cpool.tile([128, tot // 16], mybir.dt.int16)
            nc.sync.dma_start(out=v_sb[:, 0:hc], in_=vidx[:, 0:hc])
            if hc < tot // 16:
                nc.sync.dma_start(out=u_sb[:, hc:], in_=uidx[:, hc:])
                nc.sync.dma_start(out=v_sb[:, hc:], in_=vidx[:, hc:])
            scores = cpool.tile([128, totc], f32)

            for (off, n) in chunks:
                cols = n // 128

                hu = gpool.tile([128, cols * D], f16, tag="hu")
                nc.gpsimd.dma_gather(
                    hu[:].rearrange("p (s d) -> p s d", d=D),
                    tu_ap[:],
                    u_sb[:, off // 16 : (off + n) // 16],
                    n, n, D,
                    transpose=False,
                    single_packet=False,
                    queue_num=q % NQ,
                )
                q += 1
                hv = gpool.tile([128, cols * D], f16, tag="hv")
                nc.gpsimd.dma_gather(
                    hv[:].rearrange("p (s d) -> p s d", d=D),
                    tv_ap[:],
                    v_sb[:, off // 16 : (off + n) // 16],
                    n, n, D,
                    transpose=False,
                    single_packet=False,
                    queue_num=q % NQ,
                )
                q += 1

                pr = wpool.tile([128, cols * D], f16, tag="pr")
                nc.vector.tensor_mul(out=pr[:], in0=hu[:], in1=hv[:])
                # binary-tree reduce over d at 2x: 384 -> 192 -> 96 -> 48 (fp16)
                with nc.allow_low_precision("fp16 tree partial sums (3 rounding levels)"):
                    h1 = rpool.tile([128, cols * (D // 2)], f16, tag="h1")
                    p3 = pr[:].rearrange("p (s d) -> p s d", d=D)
                    nc.vector.tensor_add(
                        out=h1[:].rearrange("p (s d) -> p s d", d=D // 2),
                        in0=p3[:, :, 0 : D // 2],
                        in1=p3[:, :, D // 2 : D],
                    )
                    h2 = rpool.tile([128, cols * (D // 4)], f16, tag="h2")
                    h1r = h1[:].rearrange("p (s d) -> p s d", d=D // 2)
                    nc.vector.tensor_add(
                        out=h2[:].rearrange("p (s d) -> p s d", d=D // 4),
                        in0=h1r[:, :, 0 : D // 4],
                        in1=h1r[:, :, D // 4 : D // 2],
                    )
                    h3 = rpool.tile([128, cols * (D // 8)], f16, tag="h3")
                    h2r = h2[:].rearrange("p (s d) -> p s d", d=D // 4)
                    nc.vector.tensor_add(
                        out=h3[:].rearrange("p (s d) -> p s d", d=D // 8),
                        in0=h2r[:, :, 0 : D // 8],
                        in1=h2r[:, :, D // 8 : D // 4],
                    )
                sc = rpool.tile([128, cols], f32, tag="sc")
                nc.vector.tensor_reduce(
                    out=sc[:],
                    in_=h3[:].rearrange("p (s d) -> p s d", d=D // 8),
                    axis=mybir.AxisListType.X,
                    op=mybir.AluOpType.add,
                )
                nc.scalar.activation(
                    out=scores[:, off // 128 : off // 128 + cols],
                    in_=sc[:],
                    func=mybir.ActivationFunctionType.Sigmoid,
                )

            nc.sync.dma_start(out=out[:], in_=scores[:])

    nc.compile()
    return nc


def _get_nc(tot):
    if tot not in _cache:
        _cache[tot] = _build(tot)
    return _cache[tot]


def _wrap16(a):
    """[n] int16 -> [128, n/16] wrapped over 16 partitions, replicated 8x."""
    n = a.shape[0]
    return np.tile(a.reshape(n // 16, 16).T, (8, 1))


def _shard(u32, v32, et):
    """Shard edges by u-range; per core sort by (v, u) and build int16
    table-ordinal slot indices for the (etype,u)-pair and distinct-v
    gather tables."""
    core = u32 // USHARD
    per_core_ids = []
    max_edges = 0
    for c in range(N_CORES):
        ids = np.nonzero(core == c)[0]
        order = np.lexsort((u32[ids], v32[ids]))
        ids = ids[order]
        max_edges = max(max_edges, ids.shape[0])
        per_core_ids.append(ids)
    tot = (max_edges + SLOTPAD - 1) // SLOTPAD * SLOTPAD

    per_core = []
    for c in range(N_CORES):
        ids = per_core_ids[c]
        ne = ids.shape[0]
        # u side: distinct (etype, u) pairs
        pkey = et[ids] * N_NODES + u32[ids]
        pairs, pinv = np.unique(pkey, return_inverse=True)
        assert pairs.shape[0] <= TCAP
        # reorder table rows by first use in slot order so uidx is nearly
        # ascending -> the u-side gather reads the table near-sequentially
        npair = pairs.shape[0]
        first_pos = np.full(npair, np.iinfo(np.int64).max, np.int64)
        np.minimum.at(first_pos, pinv, np.arange(ne, dtype=np.int64))
        order = np.argsort(first_pos, kind="stable")
        rank = np.empty(npair, np.int64)
        rank[order] = np.arange(npair)
        pinv = rank[pinv]
        pairs = pairs[order]
        pu = (pairs % N_NODES).astype(np.int64)
        pk = (pairs // N_NODES).astype(np.int64)
        # v side: distinct v
        vvals, vinv = np.unique(v32[ids], return_inverse=True)
        assert vvals.shape[0] <= TCAP

        u_slots = np.zeros(tot, np.int16)
        v_slots = np.zeros(tot, np.int16)
        eid = np.full(tot, -1, np.int64)
        u_slots[:ne] = pinv.astype(np.int16)
        v_slots[:ne] = vinv.astype(np.int16)
        eid[:ne] = ids
        per_core.append((u_slots, v_slots, eid, pu, pk, vvals.astype(np.int64)))
    return tot, per_core


def _make_in_maps(h, rel_weight, per_core):
    h32 = np.asarray(h, np.float32)
    rel32 = np.asarray(rel_weight, np.float32)
    in_maps = []
    for c in range(N_CORES):
        u_slots, v_slots, _eid, pu, pk, vvals = per_core[c]
        tu = np.zeros((TCAP, D), np.float16)
        tu[: pu.shape[0]] = (h32[pu] * rel32[pk]).astype(np.float16)
        tv = np.zeros((TCAP, D), np.float16)
        tv[: vvals.shape[0]] = h32[vvals].astype(np.float16)
        in_maps.append(
            {
                "tu": tu,
                "tv": tv,
                "uidx": np.ascontiguousarray(_wrap16(u_slots)),
                "vidx": np.ascontiguousarray(_wrap16(v_slots)),
            }
        )
    return in_maps


def run_spmd(h, u, v, etype, rel_weight, trace=False, trace_cores=None):
    """Run the SPMD kernel; returns (full_output, BassKernelResults)."""
    u32 = np.asarray(u, np.int64).astype(np.int32)
    v32 = np.asarray(v, np.int64).astype(np.int32)
    et = np.asarray(etype, np.int64)
    n_edges = u32.shape[0]

    tot, per_core = _shard(u32, v32, et)
    nc = _get_nc(tot)
    in_maps = _make_in_maps(h, rel_weight, per_core)
    res = run_bass_kernel_spmd(
        nc,
        in_maps,
        core_ids=list(range(N_CORES)),
        trace=trace,
        trace_cores=trace_cores,
    )
    result = np.zeros(n_edges, np.float32)
    for c in range(N_CORES):
        o = np.asarray(res.results[c]["out"])  # [128, tot/128] f32
        vals = o.T.reshape(-1)                 # vals[s] = o[s % 128, s // 128]
        eid = per_core[c][2]
        m = eid >= 0
        result[eid[m]] = vals[m]
    return result, res


def kernel(h, u, v, etype, rel_weight):
    out, _ = run_spmd(h, u, v, etype, rel_weight)
    return out


# revision 13
# speedup vs baseline: 1.0733x; 1.0047x over previous
"""DistMult edge scoring on 8 Trainium2 NeuronCores.

score[e] = sigmoid(sum_d h[u[e],d] * rel_weight[etype[e],d] * h[v[e],d])

Strategy (v4: per-core gather tables, non-transpose gathers, DVE tree reduce)
-----------------------------------------------------------------------------
Edges are sharded across the 8 cores by u-range (core c takes u in
[12500c, 12500(c+1))) and each core's slot count is equalized (max over
cores, rounded to 128) so one SPMD program serves all cores.

Both gather sides read from small per-core tables built on the host:
  TU[j] = h[u_j] * rel_weight[k_j]   for the core's distinct (etype, u)
                                     pairs (the relation weight is folded
                                     into the u side),
  TV[j] = h[v_j]                     for the core's distinct v values.
Both tables have < 32768 rows, so slot indices are plain int16 table
ordinals -- no v-windowing, no per-etype runs, and only ~50 MB of DRAM
per core.  Slots are sorted by (v, u), making the v-side gather read
ascending (often repeated) rows -- HBM-friendly -- while the u side hits
a small (~25 MB) table.

Per 4096-slot chunk the kernel issues two *non-transposed* `dma_gather`s
(single_packet=False: coalescing ~260 descriptors into one packet
exceeds the <=64-descriptor packet ceiling and wedges the DMA).  With no
xbar in the non-transpose path, gathers rotate across all 4 SWDGE queues
so descriptor generation runs on up to 4 Q7 core pairs in parallel (it
was the serialized ~570us bottleneck of the transposed v2 design).

Gathered rows land edge-major [128, cols, 384].  One DVE fp16 multiply
(2x mode) forms prod = huR * hv; a binary-tree reduce over d via three
fp16 tensor_tensor adds (2x: 384->192->96->48) plus one 48->1
TensorReduce into fp32 (TensorReduce supports no 2x mode, so the tree
does most of the work at double rate) yields per-slot scores.  ACT
applies the sigmoid into a persistent [128, tot/128] tile DMA'd out
once at the end.
"""

import numpy as np

import concourse.bacc as bacc
import concourse.mybir as mybir
import concourse.tile as tile
from concourse.bass_utils import run_bass_kernel_spmd

N_NODES = 100000
D = 384
N_ETYPES = 8
N_CORES = 8
USHARD = N_NODES // N_CORES   # 12500 u-rows per core
SLOTPAD = 128              # slot-count granularity (layout unit)
TCAP = 32768               # gather-table rows (>= max distinct ids per core)
CH = 2048                  # slots per gather chunk
NQ = 4                     # SWDGE queues (desc-gen core pairs)

_cache = {}


def _build(tot):
    f16 = mybir.dt.float16
    f32 = mybir.dt.float32
    assert tot % SLOTPAD == 0
    totc = tot // 128

    nc = bacc.Bacc(
        "TRN2",
        target_bir_lowering=False,
        debug=False,
        enable_asserts=False,
        num_devices=N_CORES,
        num_swdge_queues=NQ,
    )
    tu_ap = nc.dram_tensor("tu", [TCAP, D], f16, kind="ExternalInput").ap()
    tv_ap = nc.dram_tensor("tv", [TCAP, D], f16, kind="ExternalInput").ap()
    uidx = nc.dram_tensor("uidx", [128, tot // 16], mybir.dt.int16, kind="ExternalInput").ap()
    vidx = nc.dram_tensor("vidx", [128, tot // 16], mybir.dt.int16, kind="ExternalInput").ap()
    out = nc.dram_tensor("out", [128, totc], f32, kind="ExternalOutput").ap()

    chunks = []
    pos = 0
    while pos < tot:
        n = min(CH, tot - pos)
        chunks.append((pos, n))
        pos += n

    q = 0
    with tile.TileContext(nc) as tc:
        with (
            tc.tile_pool(name="const", bufs=1) as cpool,
            tc.tile_pool(name="gath", bufs=5) as gpool,
            tc.tile_pool(name="prod", bufs=2) as wpool,
            tc.tile_pool(name="red", bufs=2) as rpool,
        ):
            # idx preloads split into a small head (first 2 chunks) + the
            # remainder so the first gathers' desc-gen starts immediately
            hc = min(2 * CH, tot) // 16
            u_sb = cpool.tile([128, tot // 16], mybir.dt.int16)
            nc.sync.dma_start(out=u_sb[:, 0:hc], in_=uidx[:, 0:hc])
            v_sb = cpool.tile([128, tot // 16], mybir.dt.int16)
            nc.sync.dma_start(out=v_sb[:, 0:hc], in_=vidx[:, 0:hc])
            if hc < tot // 16:
                nc.sync.dma_start(out=u_sb[:, hc:], in_=uidx[:, hc:])
                nc.sync.dma_start(out=v_sb[:, hc:], in_=vidx[:, hc:])
            scores = cpool.tile([128, totc], f32)

            for (off, n) in chunks:
                cols = n // 128

                hu = gpool.tile([128, cols * D], f16, tag="hu")
                nc.gpsimd.dma_gather(
                    hu[:].rearrange("p (s d) -> p s d", d=D),
                    tu_ap[:],
                    u_sb[:, off // 16 : (off + n) // 16],
                    n, n, D,
                    transpose=False,
                    single_packet=False,
                    queue_num=q % NQ,
                )
                q += 1
                hv = gpool.tile([128, cols * D], f16, tag="hv")
                nc.gpsimd.dma_gather(
                    hv[:].rearrange("p (s d) -> p s d", d=D),
                    tv_ap[:],
                    v_sb[:, off // 16 : (off + n) // 16],
                    n, n, D,
                    transpose=False,
                    single_packet=False,
                    queue_num=q % NQ,
                )
                q += 1

                pr = wpool.tile([128, cols * D], f16, tag="pr")
                nc.vector.tensor_mul(out=pr[:], in0=hu[:], in1=hv[:])
                # binary-tree reduce over d at 2x: 384 -> 192 -> 96 -> 48 (fp16)
                with nc.allow_low_precision("fp16 tree partial sums (3 rounding levels)"):
                    h1 = rpool.tile([128, cols * (D // 2)], f16, tag="h1")
                    p3 = pr[:].rearrange("p (s d) -> p s d", d=D)
                    nc.vector.tensor_add(
                        out=h1[:].rearrange("p (s d) -> p s d", d=D // 2),
                        in0=p3[:, :, 0 : D // 2],
                        in1=p3[:, :, D // 2 : D],
                    )
                    h2 = rpool.tile([128, cols * (D // 4)], f16, tag="h2")
                    h1r = h1[:].rearrange("p (s d) -> p s d", d=D // 2)
                    nc.vector.tensor_add(
                        out=h2[:].rearrange("p (s d) -> p s d", d=D // 4),
                        in0=h1r[:, :, 0 : D // 4],
                        in1=h1r[:, :, D // 4 : D // 2],
                    )
                    h3 = rpool.tile([128, cols * (D // 8)], f16, tag="h3")
                    h2r = h2[:].rearrange("p (s d) -> p s d", d=D // 4)
                    nc.vector.tensor_add(
                        out=h3[:].rearrange("p (s d) -> p s d", d=D // 8),
                        in0=h2r[:, :, 0 : D // 8],
                        in1=h2r[:, :, D // 8 : D // 4],
                    )
                sc = rpool.tile([128, cols], f32, tag="sc")
                nc.vector.tensor_reduce(
                    out=sc[:],
                    in_=h3[:].rearrange("p (s d) -> p s d", d=D // 8),
                    axis=mybir.AxisListType.X,
                    op=mybir.AluOpType.add,
                )
                nc.scalar.activation(
                    out=scores[:, off // 128 : off // 128 + cols],
                    in_=sc[:],
                    func=mybir.ActivationFunctionType.Sigmoid,
                )

            nc.sync.dma_start(out=out[:], in_=scores[:])

    nc.compile()
    return nc


def _get_nc(tot):
    if tot not in _cache:
        _cache[tot] = _build(tot)
    return _cache[tot]


def _wrap16(a):
    """[n] int16 -> [128, n/16] wrapped over 16 partitions, replicated 8x."""
    n = a.shape[0]
    return np.tile(a.reshape(n // 16, 16).T, (8, 1))


def _shard(u32, v32, et):
    """Shard edges by u-range; per core sort by (v, u) and build int16
    table-ordinal slot indices for the (etype,u)-pair and distinct-v
    gather tables."""
    core = u32 // USHARD
    per_core_ids = []
    max_edges = 0
    for c in range(N_CORES):
        ids = np.nonzero(core == c)[0]
        order = np.lexsort((u32[ids], v32[ids]))
        ids = ids[order]
        max_edges = max(max_edges, ids.shape[0])
        per_core_ids.append(ids)
    tot = (max_edges + SLOTPAD - 1) // SLOTPAD * SLOTPAD

    per_core = []
    for c in range(N_CORES):
        ids = per_core_ids[c]
        ne = ids.shape[0]
        # u side: distinct (etype, u) pairs
        pkey = et[ids] * N_NODES + u32[ids]
        pairs, pinv = np.unique(pkey, return_inverse=True)
        assert pairs.shape[0] <= TCAP
        # reorder table rows by first use in slot order so uidx is nearly
        # ascending -> the u-side gather reads the table near-sequentially
        npair = pairs.shape[0]
        first_pos = np.full(npair, np.iinfo(np.int64).max, np.int64)
        np.minimum.at(first_pos, pinv, np.arange(ne, dtype=np.int64))
        order = np.argsort(first_pos, kind="stable")
        rank = np.empty(npair, np.int64)
        rank[order] = np.arange(npair)
        pinv = rank[pinv]
        pairs = pairs[order]
        pu = (pairs % N_NODES).astype(np.int64)
        pk = (pairs // N_NODES).astype(np.int64)
        # v side: distinct v
        vvals, vinv = np.unique(v32[ids], return_inverse=True)
        assert vvals.shape[0] <= TCAP

        u_slots = np.zeros(tot, np.int16)
        v_slots = np.zeros(tot, np.int16)
        eid = np.full(tot, -1, np.int64)
        u_slots[:ne] = pinv.astype(np.int16)
        v_slots[:ne] = vinv.astype(np.int16)
        eid[:ne] = ids
        per_core.append((u_slots, v_slots, eid, pu, pk, vvals.astype(np.int64)))
    return tot, per_core


def _make_in_maps(h, rel_weight, per_core):
    h32 = np.asarray(h, np.float32)
    rel32 = np.asarray(rel_weight, np.float32)
    in_maps = []
    for c in range(N_CORES):
        u_slots, v_slots, _eid, pu, pk, vvals = per_core[c]
        tu = np.zeros((TCAP, D), np.float16)
        tu[: pu.shape[0]] = (h32[pu] * rel32[pk]).astype(np.float16)
        tv = np.zeros((TCAP, D), np.float16)
        tv[: vvals.shape[0]] = h32[vvals].astype(np.float16)
        in_maps.append(
            {
                "tu": tu,
                "tv": tv,
                "uidx": np.ascontiguousarray(_wrap16(u_slots)),
                "vidx": np.ascontiguousarray(_wrap16(v_slots)),
            }
        )
    return in_maps


def run_spmd(h, u, v, etype, rel_weight, trace=False, trace_cores=None):
    """Run the SPMD kernel; returns (full_output, BassKernelResults)."""
    u32 = np.asarray(u, np.int64).astype(np.int32)
    v32 = np.asarray(v, np.int64).astype(np.int32)
    et = np.asarray(etype, np.int64)
    n_edges = u32.shape[0]

    tot, per_core = _shard(u32, v32, et)
    nc = _get_nc(tot)
    in_maps = _make_in_maps(h, rel_weight, per_core)
    res = run_bass_kernel_spmd(
        nc,
        in_maps,
        core_ids=list(range(N_CORES)),
        trace=trace,
        trace_cores=trace_cores,
    )
    result = np.zeros(n_edges, np.float32)
    for c in range(N_CORES):
        o = np.asarray(res.results[c]["out"])  # [128, tot/128] f32
        vals = o.T.reshape(-1)                 # vals[s] = o[s % 128, s // 128]
        eid = per_core[c][2]
        m = eid >= 0
        result[eid[m]] = vals[m]
    return result, res


def kernel(h, u, v, etype, rel_weight):
    out, _ = run_spmd(h, u, v, etype, rel_weight)
    return out
